# revision 4
# baseline (speedup 1.0000x reference)
"""Trainium2 Bass kernel: Brill-Lindquist Christoffel symbols.

Math: the Brill-Lindquist 3-metric is conformally flat, G = psi^4 * I with
psi(y) = 1 + sum_p m_p / (2 |y - c_p|).  The Christoffel symbols collapse to

    out[b, k, i, j] = E_j d_ik + E_i d_jk - E_k d_ij,
    E_k = 2 * (d_k psi) / psi,
    d_k psi = sum_p (m_p / 2) * (c_pk - x_k) / r_p^3.

(The reference computes d_k(psi^4) by central differences with eps=1e-4; the
analytic derivative agrees with it to well below the reference's own fp32
rounding noise.)

Per point, only 3 values E_0..E_2 exist; they are scattered (with signs) into
21 of the 27 output channels, 6 channels are exactly zero.

Sharding: pure data parallel over the batch across 8 NeuronCores; masses /
centers are folded on the host into a tiny replicated constant vector.
"""

import numpy as np

import concourse.bacc as bacc
import concourse.bass as bass
import concourse.mybir as mybir
import concourse.tile as tile
from concourse.bass_utils import run_bass_kernel_spmd

AF = mybir.ActivationFunctionType
OP = mybir.AluOpType
AX = mybir.AxisListType
F32 = mybir.dt.float32

N_CORES = 8
P = 128
TILE_T = 256  # points per partition per tile
LN2 = float(np.log(2.0))


def build_program(bc: int, tile_t: int = TILE_T):
    """Bass program for one core processing `bc` points (SPMD across cores)."""
    npp = bc // P
    T = tile_t
    ntiles = npp // T
    assert bc == npp * P and npp == ntiles * T

    nc = bacc.Bacc(None, target_bir_lowering=False)
    x = nc.dram_tensor("x", [bc, 3], F32, kind="ExternalInput")
    consts = nc.dram_tensor("consts", [P, 16], F32, kind="ExternalInput")
    out = nc.dram_tensor("out", [bc, 27], F32, kind="ExternalOutput")
    xv = x.rearrange("(p n) k -> p n k", p=P)
    ov = out.rearrange("(p n) c -> p n c", p=P)

    def chans(ot, base, step, count):
        # [P, T, count] view over output channels {base, base+step, ...}
        return bass.AP(
            tensor=ot.tensor,
            offset=ot.offset + base,
            ap=[ot.ap[0], [27, T], [step, count]],
        )

    with tile.TileContext(nc) as tc:
        with (
            tc.tile_pool(name="cpool", bufs=1) as cpool,
            tc.tile_pool(name="xpool", bufs=3) as xpool,
            tc.tile_pool(name="opool", bufs=1) as opool,
            tc.tile_pool(name="wpool", bufs=2) as wpool,
        ):
            ct = cpool.tile([P, 16], F32)
            nc.sync.dma_start(out=ct[:], in_=consts[:])
            c0b = ct[:, 0:3].unsqueeze(1).broadcast_to([P, T, 3])
            c1b = ct[:, 3:6].unsqueeze(1).broadcast_to([P, T, 3])

            # Persistent output tiles; the 6 always-zero channels are zeroed
            # once here and never rewritten.
            n_ot = min(3, ntiles)
            otiles = []
            for j in range(n_ot):
                ot = opool.tile([P, T, 27], F32, tag=f"ot{j}", name=f"ot{j}")
                nc.gpsimd.memset(chans(ot, 5, 16, 2), 0.0)
                nc.gpsimd.memset(chans(ot, 7, 4, 4), 0.0)
                otiles.append(ot)

            for i in range(ntiles):
                xin = xpool.tile([P, T, 3], F32)
                nc.sync.dma_start(out=xin[:], in_=xv[:, i * T:(i + 1) * T, :])

                dx0 = wpool.tile([P, T, 3], F32)
                nc.vector.tensor_tensor(dx0[:], c0b, xin[:], OP.subtract)
                dx1 = wpool.tile([P, T, 3], F32)
                nc.vector.tensor_tensor(dx1[:], c1b, xin[:], OP.subtract)

                sq0 = wpool.tile([P, T, 3], F32)
                nc.scalar.activation(sq0[:], dx0[:], AF.Square, bias=ct[:, 10:11])
                sq1 = wpool.tile([P, T, 3], F32)
                nc.scalar.activation(sq1[:], dx1[:], AF.Square, bias=ct[:, 10:11])

                s0 = wpool.tile([P, T], F32)
                nc.vector.tensor_reduce(s0[:], sq0[:], axis=AX.X, op=OP.add)
                s1 = wpool.tile([P, T], F32)
                nc.vector.tensor_reduce(s1[:], sq1[:], axis=AX.X, op=OP.add)

                # ln_p = Ln(a_p * s_p);  u_p = (a_p s_p)^-1/2 = m_p/(2 r_p)
                # t_p = a_p * (a_p s_p)^-3/2 = (m_p/2) * r_p^-3
                ln0 = wpool.tile([P, T], F32)
                nc.scalar.activation(ln0[:], s0[:], AF.Ln, bias=ct[:, 10:11], scale=ct[:, 6:7])
                ln1 = wpool.tile([P, T], F32)
                nc.scalar.activation(ln1[:], s1[:], AF.Ln, bias=ct[:, 10:11], scale=ct[:, 7:8])
                u0 = wpool.tile([P, T], F32)
                nc.scalar.activation(u0[:], ln0[:], AF.Exp, bias=ct[:, 10:11], scale=-0.5)
                u1 = wpool.tile([P, T], F32)
                nc.scalar.activation(u1[:], ln1[:], AF.Exp, bias=ct[:, 10:11], scale=-0.5)
                t0 = wpool.tile([P, T], F32)
                nc.scalar.activation(t0[:], ln0[:], AF.Exp, bias=ct[:, 8:9], scale=-1.5)
                t1 = wpool.tile([P, T], F32)
                nc.scalar.activation(t1[:], ln1[:], AF.Exp, bias=ct[:, 9:10], scale=-1.5)

                # w2 = 2 / psi, psi = 1 + u0 + u1
                v = wpool.tile([P, T], F32)
                nc.vector.tensor_tensor(v[:], u0[:], u1[:], OP.add)
                lnp = wpool.tile([P, T], F32)
                nc.scalar.activation(lnp[:], v[:], AF.Ln, bias=ct[:, 11:12])
                w2 = wpool.tile([P, T], F32)
                nc.scalar.activation(w2[:], lnp[:], AF.Exp, bias=ct[:, 12:13], scale=-1.0)

                # grad_k = t0*(c0-x)_k + t1*(c1-x)_k = d_k psi
                mul0 = wpool.tile([P, T, 3], F32)
                nc.vector.tensor_tensor(
                    mul0[:], dx0[:], t0.unsqueeze(2).broadcast_to([P, T, 3]), OP.mult
                )
                mul1 = wpool.tile([P, T, 3], F32)
                nc.vector.tensor_tensor(
                    mul1[:], dx1[:], t1.unsqueeze(2).broadcast_to([P, T, 3]), OP.mult
                )
                grad = wpool.tile([P, T, 3], F32)
                nc.vector.tensor_tensor(grad[:], mul0[:], mul1[:], OP.add)

                ot = otiles[i % n_ot]
                # [E0,E1,E2] -> channels {0,1,2}; replicate to {12..14}, {24..26}
                nc.vector.tensor_tensor(
                    chans(ot, 0, 1, 3),
                    grad[:],
                    w2.unsqueeze(2).broadcast_to([P, T, 3]),
                    OP.mult,
                )
                nc.vector.tensor_copy(chans(ot, 12, 1, 3), chans(ot, 0, 1, 3))
                nc.scalar.copy(chans(ot, 24, 1, 3), chans(ot, 0, 1, 3))
                E0 = chans(ot, 0, 0, 2)
                E1 = chans(ot, 1, 0, 2)
                E2 = chans(ot, 2, 0, 2)
                # remaining 12 nonzero channels as 6 paired strided copies
                nc.vector.tensor_scalar(chans(ot, 4, 4, 2), E0, -1.0, None, OP.mult)
                nc.vector.tensor_copy(chans(ot, 10, 10, 2), E0)
                nc.vector.tensor_scalar(chans(ot, 18, 4, 2), E2, -1.0, None, OP.mult)
                nc.scalar.copy(chans(ot, 3, 20, 2), E1)
                nc.scalar.mul(chans(ot, 9, 8, 2), E1, -1.0)
                nc.scalar.copy(chans(ot, 6, 10, 2), E2)

                nc.sync.dma_start(out=ov[:, i * T:(i + 1) * T, :], in_=ot[:])
    nc.compile()
    return nc


_programs: dict = {}


def _get_program(bc: int):
    if bc not in _programs:
        _programs[bc] = build_program(bc)
    return _programs[bc]


def make_consts(masses: np.ndarray, centers: np.ndarray) -> np.ndarray:
    m = np.asarray(masses, dtype=np.float64)
    c = np.asarray(centers, dtype=np.float64)
    a = 4.0 / m**2
    b = np.log(a)
    vec = np.concatenate(
        [c[0], c[1], a, b, [0.0, 1.0, np.log(2.0), 0.0, 0.0, 0.0]]
    ).astype(np.float32)  # [16]
    return np.tile(vec[None, :], (P, 1))


def kernel(x, masses, centers):
    x = np.ascontiguousarray(np.asarray(x, dtype=np.float32))
    B = x.shape[0]
    assert B % N_CORES == 0
    bc = B // N_CORES
    nc = _get_program(bc)
    consts = make_consts(masses, centers)
    in_maps = [
        {"x": x[i * bc:(i + 1) * bc], "consts": consts} for i in range(N_CORES)
    ]
    res = run_bass_kernel_spmd(nc, in_maps, core_ids=list(range(N_CORES)))
    full = np.concatenate([res.results[i]["out"] for i in range(N_CORES)], axis=0)
    return full.reshape(B, 3, 3, 3)


# revision 6
# speedup vs baseline: 157.6899x; 157.6899x over previous
"""Trainium2 Bass kernel: Brill-Lindquist Christoffel symbols.

Math: the Brill-Lindquist 3-metric is conformally flat, G = psi^4 * I with
psi(y) = 1 + sum_p m_p / (2 |y - c_p|).  The Christoffel symbols collapse to

    out[b, k, i, j] = E_j d_ik + E_i d_jk - E_k d_ij,
    E_k = 2 * (d_k psi) / psi,
    d_k psi = sum_p (m_p / 2) * (c_pk - x_k) / r_p^3.

(The reference computes d_k(psi^4) by central differences with eps=1e-4; the
analytic derivative agrees with it to well below the reference's own fp32
rounding noise.)

Per point, only 3 values E_0..E_2 exist; they are scattered (with signs) into
21 of the 27 output channels, 6 channels are exactly zero.

Sharding: pure data parallel over the batch across 8 NeuronCores; masses /
centers are folded on the host into a tiny replicated constant vector.
"""

import numpy as np

import concourse.bacc as bacc
import concourse.bass as bass
import concourse.mybir as mybir
import concourse.tile as tile
from concourse.bass_utils import run_bass_kernel_spmd

AF = mybir.ActivationFunctionType
OP = mybir.AluOpType
AX = mybir.AxisListType
F32 = mybir.dt.float32

N_CORES = 8
P = 128
TILE_T = 256  # points per partition per tile
LN2 = float(np.log(2.0))


def build_program(bc: int, tile_t: int = TILE_T, reps: int = 1):
    """Bass program for one core processing `bc` points (SPMD across cores).

    reps > 1 repeats the whole tile loop (same data, same outputs) — used only
    by the perf harness to measure steady-state kernel time above the
    constant dispatch overhead.
    """
    npp = bc // P
    T = tile_t
    ntiles = npp // T
    assert bc == npp * P and npp == ntiles * T

    nc = bacc.Bacc(None, target_bir_lowering=False)
    x = nc.dram_tensor("x", [bc, 3], F32, kind="ExternalInput")
    consts = nc.dram_tensor("consts", [P, 16], F32, kind="ExternalInput")
    out = nc.dram_tensor("out", [bc, 27], F32, kind="ExternalOutput")
    xv = x.rearrange("(p n) k -> p n k", p=P)
    ov = out.rearrange("(p n) c -> p n c", p=P)

    def chans(ot, base, step, count):
        # [P, T, count] view over output channels {base, base+step, ...}
        return bass.AP(
            tensor=ot.tensor,
            offset=ot.offset + base,
            ap=[ot.ap[0], [27, T], [step, count]],
        )

    with tile.TileContext(nc) as tc:
        with (
            tc.tile_pool(name="cpool", bufs=1) as cpool,
            tc.tile_pool(name="xpool", bufs=3) as xpool,
            tc.tile_pool(name="opool", bufs=1) as opool,
            tc.tile_pool(name="wpool", bufs=2) as wpool,
        ):
            ct = cpool.tile([P, 16], F32)
            nc.sync.dma_start(out=ct[:], in_=consts[:])
            c0b = ct[:, 0:3].unsqueeze(1).broadcast_to([P, T, 3])
            c1b = ct[:, 3:6].unsqueeze(1).broadcast_to([P, T, 3])

            # Persistent output tiles; the 6 always-zero channels are zeroed
            # once here and never rewritten.
            n_ot = min(3, ntiles)
            otiles = []
            for j in range(n_ot):
                ot = opool.tile([P, T, 27], F32, tag=f"ot{j}", name=f"ot{j}")
                nc.gpsimd.memset(chans(ot, 5, 16, 2), 0.0)
                nc.gpsimd.memset(chans(ot, 7, 4, 4), 0.0)
                otiles.append(ot)

            for i in range(ntiles * reps):
                i = i % ntiles
                xin = xpool.tile([P, T, 3], F32)
                nc.sync.dma_start(out=xin[:], in_=xv[:, i * T:(i + 1) * T, :])

                dx0 = wpool.tile([P, T, 3], F32)
                nc.vector.tensor_tensor(dx0[:], c0b, xin[:], OP.subtract)
                dx1 = wpool.tile([P, T, 3], F32)
                nc.vector.tensor_tensor(dx1[:], c1b, xin[:], OP.subtract)

                sq0 = wpool.tile([P, T, 3], F32)
                nc.scalar.activation(sq0[:], dx0[:], AF.Square, bias=ct[:, 10:11])
                sq1 = wpool.tile([P, T, 3], F32)
                nc.scalar.activation(sq1[:], dx1[:], AF.Square, bias=ct[:, 10:11])

                s0 = wpool.tile([P, T], F32)
                nc.vector.tensor_reduce(s0[:], sq0[:], axis=AX.X, op=OP.add)
                s1 = wpool.tile([P, T], F32)
                nc.vector.tensor_reduce(s1[:], sq1[:], axis=AX.X, op=OP.add)

                # ln_p = Ln(a_p * s_p);  u_p = (a_p s_p)^-1/2 = m_p/(2 r_p)
                # t_p = a_p * (a_p s_p)^-3/2 = (m_p/2) * r_p^-3
                ln0 = wpool.tile([P, T], F32)
                nc.scalar.activation(ln0[:], s0[:], AF.Ln, bias=ct[:, 10:11], scale=ct[:, 6:7])
                ln1 = wpool.tile([P, T], F32)
                nc.scalar.activation(ln1[:], s1[:], AF.Ln, bias=ct[:, 10:11], scale=ct[:, 7:8])
                u0 = wpool.tile([P, T], F32)
                nc.scalar.activation(u0[:], ln0[:], AF.Exp, bias=ct[:, 10:11], scale=-0.5)
                u1 = wpool.tile([P, T], F32)
                nc.scalar.activation(u1[:], ln1[:], AF.Exp, bias=ct[:, 10:11], scale=-0.5)
                t0 = wpool.tile([P, T], F32)
                nc.scalar.activation(t0[:], ln0[:], AF.Exp, bias=ct[:, 8:9], scale=-1.5)
                t1 = wpool.tile([P, T], F32)
                nc.scalar.activation(t1[:], ln1[:], AF.Exp, bias=ct[:, 9:10], scale=-1.5)

                # w2 = 2 / psi, psi = 1 + u0 + u1
                v = wpool.tile([P, T], F32)
                nc.vector.tensor_tensor(v[:], u0[:], u1[:], OP.add)
                lnp = wpool.tile([P, T], F32)
                nc.scalar.activation(lnp[:], v[:], AF.Ln, bias=ct[:, 11:12])
                w2 = wpool.tile([P, T], F32)
                nc.scalar.activation(w2[:], lnp[:], AF.Exp, bias=ct[:, 12:13], scale=-1.0)

                # grad_k = t0*(c0-x)_k + t1*(c1-x)_k = d_k psi
                mul0 = wpool.tile([P, T, 3], F32)
                nc.vector.tensor_tensor(
                    mul0[:], dx0[:], t0.unsqueeze(2).broadcast_to([P, T, 3]), OP.mult
                )
                mul1 = wpool.tile([P, T, 3], F32)
                nc.vector.tensor_tensor(
                    mul1[:], dx1[:], t1.unsqueeze(2).broadcast_to([P, T, 3]), OP.mult
                )
                grad = wpool.tile([P, T, 3], F32)
                nc.vector.tensor_tensor(grad[:], mul0[:], mul1[:], OP.add)

                ot = otiles[i % n_ot]
                # [E0,E1,E2] -> channels {0,1,2}; replicate to {12..14}, {24..26}
                nc.vector.tensor_tensor(
                    chans(ot, 0, 1, 3),
                    grad[:],
                    w2.unsqueeze(2).broadcast_to([P, T, 3]),
                    OP.mult,
                )
                nc.vector.tensor_copy(chans(ot, 12, 1, 3), chans(ot, 0, 1, 3))
                nc.scalar.copy(chans(ot, 24, 1, 3), chans(ot, 0, 1, 3))
                E0 = chans(ot, 0, 0, 2)
                E1 = chans(ot, 1, 0, 2)
                E2 = chans(ot, 2, 0, 2)
                # remaining 12 nonzero channels as 6 paired strided copies
                nc.vector.tensor_scalar(chans(ot, 4, 4, 2), E0, -1.0, None, OP.mult)
                nc.vector.tensor_copy(chans(ot, 10, 10, 2), E0)
                nc.vector.tensor_scalar(chans(ot, 18, 4, 2), E2, -1.0, None, OP.mult)
                nc.scalar.copy(chans(ot, 3, 20, 2), E1)
                nc.scalar.mul(chans(ot, 9, 8, 2), E1, -1.0)
                nc.scalar.copy(chans(ot, 6, 10, 2), E2)

                nc.sync.dma_start(out=ov[:, i * T:(i + 1) * T, :], in_=ot[:])
    nc.compile()
    return nc


_programs: dict = {}


def _get_program(bc: int):
    if bc not in _programs:
        _programs[bc] = build_program(bc)
    return _programs[bc]


def make_consts(masses: np.ndarray, centers: np.ndarray) -> np.ndarray:
    m = np.asarray(masses, dtype=np.float64)
    c = np.asarray(centers, dtype=np.float64)
    a = 4.0 / m**2
    b = np.log(a)
    vec = np.concatenate(
        [c[0], c[1], a, b, [0.0, 1.0, np.log(2.0), 0.0, 0.0, 0.0]]
    ).astype(np.float32)  # [16]
    return np.tile(vec[None, :], (P, 1))


def kernel(x, masses, centers):
    x = np.ascontiguousarray(np.asarray(x, dtype=np.float32))
    B = x.shape[0]
    assert B % N_CORES == 0
    bc = B // N_CORES
    nc = _get_program(bc)
    consts = make_consts(masses, centers)
    in_maps = [
        {"x": x[i * bc:(i + 1) * bc], "consts": consts} for i in range(N_CORES)
    ]
    res = run_bass_kernel_spmd(nc, in_maps, core_ids=list(range(N_CORES)))
    full = np.concatenate([res.results[i]["out"] for i in range(N_CORES)], axis=0)
    return full.reshape(B, 3, 3, 3)
